# revision 9
# baseline (speedup 1.0000x reference)
"""Trainium2 Bass/Tile kernel for a GPT-style transformer block.

reference semantics (B=128, T=256, C=384, H=6 heads, FF=1536):
    h  = LN(x; g1, be1)
    x2 = x + CausalAttention(h; Wk,Wq,Wv,Wo,bo)
    h2 = LN(x2; g2, be2)
    out = x2 + (relu(h2 @ W1 + b1) @ W2 + b2)

Sharding: pure data-parallel over batch across 8 NeuronCores (16 batch
elements per core), one SPMD Bass program, no collectives.

Kernel dataflow (per core, per pair of batch elements):
  - x loaded in natural (token-partition) layout; LayerNorm stats via
    bn_stats/bn_aggr; normalized z cast to bf16.
  - z transposed 128x128-blockwise on the PE into z^T (C on partitions).
  - Q^T/K^T = Wq'^T @ z^T, V natural = z @ Wv' (bf16 matmuls, fp32 PSUM).
  - Per head: S^T = K_h @ Q_h^T (keys on partitions, queries free),
    E^T = exp(S^T/8) * causal_mask (exp on ACT straight out of PSUM,
    triangle mask-mul on DVE; fully-masked blocks never computed).
  - U^T = [V_h | 1]^T @ E^T -> numerator rows 0:64 + denominator row 64.
  - O^T = U^T * bcast(1/denom): reciprocal on DVE, broadcast across
    partitions via a tiny rank-2 PE matmul (float32r), divide on DVE.
  - Y = O^T.T @ Wo (natural layout), residual add, LN2, FFN with
    fc1 in transposed form (relu fused into the PSUM->SBUF copy),
    fc2 back to natural, final residual, DMA out.

All (nonzero) affine parameters are folded host-side:
    Wq' = diag(g1) Wq (same k/v), bq = be1 @ Wq (per-partition in Q^T), ...
    W1' = diag(g2) W1, b1' = b1 + be2 @ W1 (per-partition in fc1^T).
bo / (be1 @ Wv) / b2 are free-dim biases in their layouts; they are
zero for this problem's inputs and emitted only if nonzero (via rank-1
ones matmuls into the accumulating PSUM).
"""

import numpy as np
import ml_dtypes

import concourse.bass as bass
import concourse.bacc as bacc
import concourse.tile as tile
from concourse import mybir
from concourse import bass_utils

B, T, C = 128, 256, 384
H, D = 6, 64
FF = 1536
EPS = 1e-5
NCORES = 8
BL = B // NCORES          # 16 batch elements per core
NPAIRS = BL // 2          # processed two at a time
KC = C // 128             # 3 contraction chunks over C
FC = FF // 128            # 12 chunks over FF

F32 = mybir.dt.float32
BF16 = mybir.dt.bfloat16
F32R = mybir.dt.float32r
AF = mybir.ActivationFunctionType
ALU = mybir.AluOpType

bf16 = ml_dtypes.bfloat16

_built = {}


def _build(flags):
    """Build + compile the SPMD Bass program."""
    has_qkb, has_b1, has_vb, has_bo, has_b2 = flags
    nc = bacc.Bacc("TRN2", debug=False, target_bir_lowering=False,
                   num_devices=NCORES)

    x_d = nc.dram_tensor("x", [BL * T, C], F32, kind="ExternalInput").ap()
    out_d = nc.dram_tensor("out", [BL * T, C], F32, kind="ExternalOutput").ap()
    wq_d = nc.dram_tensor("wq", [C, C], BF16, kind="ExternalInput").ap()
    wk_d = nc.dram_tensor("wk", [C, C], BF16, kind="ExternalInput").ap()
    wv_d = nc.dram_tensor("wv", [C, C], BF16, kind="ExternalInput").ap()
    wo_d = nc.dram_tensor("wo", [C, C], BF16, kind="ExternalInput").ap()
    w1_d = nc.dram_tensor("w1", [C, FF], BF16, kind="ExternalInput").ap()
    w2_d = nc.dram_tensor("w2", [FF, C], BF16, kind="ExternalInput").ap()
    bq_d = nc.dram_tensor("bq", [128, KC], F32, kind="ExternalInput").ap()
    bk_d = nc.dram_tensor("bk", [128, KC], F32, kind="ExternalInput").ap()
    b1_d = nc.dram_tensor("b1p", [128, FC], F32, kind="ExternalInput").ap()
    tri_d = nc.dram_tensor("tri", [128, 128], BF16, kind="ExternalInput").ap()
    idn_d = nc.dram_tensor("iden", [128, 128], BF16, kind="ExternalInput").ap()
    sel_d = nc.dram_tensor("sel2", [1, 128], BF16, kind="ExternalInput").ap()
    if has_vb:
        vb_d = nc.dram_tensor("vbrow", [1, C], BF16, kind="ExternalInput").ap()
    if has_bo:
        bo_d = nc.dram_tensor("borow", [1, C], BF16, kind="ExternalInput").ap()
    if has_b2:
        b2_d = nc.dram_tensor("b2row", [1, C], BF16, kind="ExternalInput").ap()
    if has_vb or has_bo or has_b2:
        ones_d = nc.dram_tensor("ones1", [1, 128], BF16,
                                kind="ExternalInput").ap()

    ctx_lp = nc.allow_low_precision(reason="bf16 softmax denominators")
    ctx_lp.__enter__()
    with tile.TileContext(nc) as tc:
        with (
            tc.tile_pool(name="consts", bufs=1) as cp,
            tc.tile_pool(name="zT", bufs=4) as zTp,
            tc.tile_pool(name="qkT", bufs=8) as qkTp,
            tc.tile_pool(name="vt", bufs=6) as vtp,
            tc.tile_pool(name="ep", bufs=6) as ep,
            tc.tile_pool(name="oT", bufs=4) as oTp,
            tc.tile_pool(name="xin", bufs=8) as xp,
            tc.tile_pool(name="x2", bufs=8) as x2p,
            tc.tile_pool(name="zz", bufs=3) as zp,
            tc.tile_pool(name="f1r", bufs=14) as f1p,
            tc.tile_pool(name="osb", bufs=4) as op,
            tc.tile_pool(name="st", bufs=8) as sp,
            tc.tile_pool(name="rb", bufs=4) as rbp,
            # PSUM: 8 banks total.  pBig: all accumulation outputs
            # (qk/v/y/fc1/fc2).  pS: scores + transposes + recip bcast.
            # pU: per-head numerator/denominator.
            tc.tile_pool(name="pBig", bufs=3, space="PSUM") as pBig,
            tc.tile_pool(name="pS", bufs=3, space="PSUM") as pSp,
            tc.tile_pool(name="pU", bufs=2, space="PSUM") as pUp,
        ):
            # ---- constants / weights resident in SBUF ----
            wq = [cp.tile([128, C], BF16, tag=f"wq{k}", name=f"wq{k}") for k in range(KC)]
            wk = [cp.tile([128, C], BF16, tag=f"wk{k}", name=f"wk{k}") for k in range(KC)]
            wv = [cp.tile([128, C], BF16, tag=f"wv{k}", name=f"wv{k}") for k in range(KC)]
            wo = [cp.tile([128, C], BF16, tag=f"wo{k}", name=f"wo{k}") for k in range(KC)]
            w1 = [cp.tile([128, FF], BF16, tag=f"w1{k}", name=f"w1{k}") for k in range(KC)]
            w2 = [cp.tile([128, C], BF16, tag=f"w2{k}", name=f"w2{k}") for k in range(FC)]
            for k in range(KC):
                nc.sync.dma_start(wq[k][:], wq_d[128 * k:128 * (k + 1), :])
                nc.sync.dma_start(wk[k][:], wk_d[128 * k:128 * (k + 1), :])
                nc.sync.dma_start(wv[k][:], wv_d[128 * k:128 * (k + 1), :])
                nc.sync.dma_start(wo[k][:], wo_d[128 * k:128 * (k + 1), :])
                nc.sync.dma_start(w1[k][:], w1_d[128 * k:128 * (k + 1), :])
            for k in range(FC):
                nc.sync.dma_start(w2[k][:], w2_d[128 * k:128 * (k + 1), :])
            bq = cp.tile([128, KC], F32, tag="bq", name="bq")
            bk = cp.tile([128, KC], F32, tag="bk", name="bk")
            b1 = cp.tile([128, FC], F32, tag="b1", name="b1")
            tri = cp.tile([128, 128], BF16, tag="tri", name="tri")
            idn = cp.tile([128, 128], BF16, tag="idn", name="idn")
            sel = cp.tile([1, 128], BF16, tag="sel", name="sel")
            epst = cp.tile([128, 1], F32, tag="eps", name="eps")
            nc.sync.dma_start(bq[:], bq_d[:])
            nc.sync.dma_start(bk[:], bk_d[:])
            nc.sync.dma_start(b1[:], b1_d[:])
            nc.sync.dma_start(tri[:], tri_d[:])
            nc.sync.dma_start(idn[:], idn_d[:])
            nc.sync.dma_start(sel[:], sel_d[:])
            nc.vector.memset(epst[:], EPS)
            vb = bo = b2 = on1 = None
            if has_vb:
                vb = cp.tile([1, C], BF16, tag="vb", name="vb")
                nc.sync.dma_start(vb[:], vb_d[:])
            if has_bo:
                bo = cp.tile([1, C], BF16, tag="bo", name="bo")
                nc.sync.dma_start(bo[:], bo_d[:])
            if has_b2:
                b2 = cp.tile([1, C], BF16, tag="b2", name="b2")
                nc.sync.dma_start(b2[:], b2_d[:])
            if has_vb or has_bo or has_b2:
                on1 = cp.tile([1, 128], BF16, tag="on1", name="on1")
                nc.sync.dma_start(on1[:], ones_d[:])

            def layernorm_T(xt_tiles, ztag, zTtag):
                """4 natural (128,C) f32 tiles -> KC (128,512) bf16 z^T tiles
                (C on partitions, pair-tokens on free)."""
                zT = [zTp.tile([128, 512], BF16, tag=zTtag, name=zTtag)
                      for _ in range(KC)]
                for tt in range(4):
                    xt = xt_tiles[tt]
                    st6 = sp.tile([128, 6], F32, tag="bn6", name="bn6")
                    mv = sp.tile([128, 2], F32, tag="mv", name="mv")
                    rstd = sp.tile([128, 1], F32, tag="rstd", name="rstd")
                    nc.vector.bn_stats(out=st6[:], in_=xt[:])
                    nc.vector.bn_aggr(out=mv[:], in_=st6[:])
                    nc.scalar.activation(out=rstd[:], in_=mv[:, 1:2],
                                         func=AF.Sqrt, bias=epst[:])
                    nc.vector.reciprocal(out=rstd[:], in_=rstd[:])
                    z = zp.tile([128, C], BF16, tag=ztag)
                    nc.vector.tensor_scalar(
                        out=z[:], in0=xt[:], scalar1=mv[:, 0:1],
                        scalar2=rstd[:], op0=ALU.subtract, op1=ALU.mult)
                    for k in range(KC):
                        pt = pSp.tile([128, 128], BF16, tag="ps", name="ps")
                        nc.tensor.transpose(pt[:], z[:, 128 * k:128 * (k + 1)],
                                            idn[:])
                        nc.vector.tensor_copy(
                            zT[k][:, 128 * tt:128 * (tt + 1)], pt[:])
                return zT

            for pair in range(NPAIRS):
                # ---- load x (4 token tiles of (128, C)) ----
                xt = []
                for tt in range(4):
                    t_ = xp.tile([128, C], F32, tag="x", name="x")
                    r0 = pair * 2 * T + tt * 128
                    nc.sync.dma_start(t_[:], x_d[r0:r0 + 128, :])
                    xt.append(t_)

                zT = layernorm_T(xt, "z1", "z1T")

                # ---- Q^T / K^T (C_out on partitions, pair tokens free) ----
                qT, kT = [], []
                for (wmat, bias, dst) in ((wq, bq, qT), (wk, bk, kT)):
                    for m in range(KC):
                        ps = pBig.tile([128, 512], F32, tag="big", name="big")
                        for k in range(KC):
                            nc.tensor.matmul(
                                ps[:], wmat[k][:, 128 * m:128 * (m + 1)],
                                zT[k][:], start=(k == 0), stop=(k == KC - 1))
                        t_ = qkTp.tile([128, 512], BF16, tag="qkT", name="qkT")
                        if has_qkb:
                            nc.scalar.activation(out=t_[:], in_=ps[:],
                                                 func=AF.Identity,
                                                 bias=bias[:, m:m + 1])
                        else:
                            nc.scalar.activation(out=t_[:], in_=ps[:],
                                                 func=AF.Copy)
                        dst.append(t_)

                # ---- V natural, with interleaved ones column per head ----
                vt = []
                for tt in range(4):
                    ps = pBig.tile([128, C], F32, tag="big", name="big")
                    for k in range(KC):
                        nc.tensor.matmul(
                            ps[:], zT[k][:, 128 * tt:128 * (tt + 1)],
                            wv[k][:], start=(k == 0),
                            stop=(k == KC - 1 and not has_vb))
                    if has_vb:
                        nc.tensor.matmul(ps[:], on1[:], vb[:],
                                         start=False, stop=True)
                    t_ = vtp.tile([128, H * 65], BF16, tag="vt", name="vt")
                    t3 = t_.rearrange("p (h e) -> p h e", e=65)
                    nc.vector.memset(t3[:, :, 64:65], 1.0)
                    nc.scalar.activation(
                        out=t3[:, :, 0:64],
                        in_=ps[:].rearrange("p (h e) -> p h e", e=64),
                        func=AF.Copy)
                    vt.append(t_)

                # ---- attention, transposed domain, per element/head ----
                oT = [oTp.tile([128, 512], BF16, tag="oT", name="oT")
                      for _ in range(KC)]
                for e in range(2):
                    es = 256 * e
                    v0 = vt[2 * e]
                    v1 = vt[2 * e + 1]
                    for c in range(KC):
                        rb = rbp.tile([1, 512], BF16, tag="rb", name="rb")
                        upair = []
                        for hh in range(2):
                            h = 2 * c + hh
                            po = hh * 64
                            q_h = qT[c][po:po + 64, es:es + 256]
                            k_h = kT[c][po:po + 64, es:es + 256]
                            # S^T: keys on partitions, queries free
                            s0 = pSp.tile([128, 256], F32, tag="ps", name="ps")
                            s1 = pSp.tile([128, 128], F32, tag="ps", name="ps")
                            nc.tensor.matmul(s0[:], k_h[:, 0:128], q_h,
                                             start=True, stop=True)
                            nc.tensor.matmul(s1[:], k_h[:, 128:256],
                                             q_h[:, 128:256],
                                             start=True, stop=True)
                            e0 = ep.tile([128, 256], BF16, tag="e0", name="e0")
                            e1 = ep.tile([128, 128], BF16, tag="e1", name="e1")
                            nc.scalar.activation(out=e0[:], in_=s0[:],
                                                 func=AF.Exp, scale=0.125)
                            nc.scalar.activation(out=e1[:], in_=s1[:],
                                                 func=AF.Exp, scale=0.125)
                            nc.vector.tensor_mul(e0[:, 0:128],
                                                 e0[:, 0:128], tri[:])
                            nc.vector.tensor_mul(e1[:], e1[:], tri[:])
                            # U^T (64 rows) + denominator (row 64)
                            u = pUp.tile([65, 256], F32, tag="u", name="u")
                            nc.tensor.matmul(
                                u[:], v0[:, 65 * h:65 * (h + 1)], e0[:],
                                start=True, stop=False)
                            nc.tensor.matmul(
                                u[0:65, 128:256],
                                v1[:, 65 * h:65 * (h + 1)], e1[:],
                                start=False, stop=True)
                            nc.vector.reciprocal(
                                out=rb[0:1, 256 * hh:256 * (hh + 1)],
                                in_=u[64:65, :])
                            upair.append(u)
                        # broadcast 1/denom across all partitions
                        pr = pSp.tile([128, 512], F32, tag="ps", name="ps")
                        nc.tensor.matmul(pr[:], sel[:], rb[:],
                                         start=True, stop=True)
                        rbs = rbp.tile([128, 512], F32, tag="rbs", name="rbs")
                        nc.scalar.activation(out=rbs[:], in_=pr[:],
                                             func=AF.Copy)
                        nc.vector.tensor_mul(oT[c][0:64, es:es + 256],
                                             upair[0][0:64, :],
                                             rbs[0:64, 0:256])
                        nc.vector.tensor_mul(oT[c][64:128, es:es + 256],
                                             upair[1][0:64, :],
                                             rbs[64:128, 256:512])

                # ---- Y = O @ Wo (natural), residual, LN2 ----
                x2t = []
                for tt in range(4):
                    ps = pBig.tile([128, C], F32, tag="big", name="big")
                    for k in range(KC):
                        nc.tensor.matmul(
                            ps[:], oT[k][:, 128 * tt:128 * (tt + 1)],
                            wo[k][:], start=(k == 0),
                            stop=(k == KC - 1 and not has_bo))
                    if has_bo:
                        nc.tensor.matmul(ps[:], on1[:], bo[:],
                                         start=False, stop=True)
                    x2 = x2p.tile([128, C], F32, tag="x2", name="x2")
                    nc.vector.tensor_add(x2[:], ps[:], xt[tt][:])
                    x2t.append(x2)

                z2T = layernorm_T(x2t, "z2", "z2T")

                # ---- FFN: fc1 transposed (relu fused), fc2 natural ----
                f1r = []
                for m in range(FC):
                    ps = pBig.tile([128, 512], F32, tag="big", name="big")
                    for k in range(KC):
                        nc.tensor.matmul(
                            ps[:], w1[k][:, 128 * m:128 * (m + 1)],
                            z2T[k][:], start=(k == 0), stop=(k == KC - 1))
                    t_ = f1p.tile([128, 512], BF16, tag="f1r", name="f1r")
                    nc.scalar.activation(out=t_[:], in_=ps[:], func=AF.Relu,
                                         bias=(b1[:, m:m + 1] if has_b1
                                               else 0.0))
                    f1r.append(t_)

                for tt in range(4):
                    ps = pBig.tile([128, C], F32, tag="big", name="big")
                    for k in range(FC):
                        nc.tensor.matmul(
                            ps[:], f1r[k][:, 128 * tt:128 * (tt + 1)],
                            w2[k][:], start=(k == 0),
                            stop=(k == FC - 1 and not has_b2))
                    if has_b2:
                        nc.tensor.matmul(ps[:], on1[:], b2[:],
                                         start=False, stop=True)
                    ot = op.tile([128, C], F32, tag="ot", name="ot")
                    nc.vector.tensor_add(ot[:], ps[:], x2t[tt][:])
                    r0 = pair * 2 * T + tt * 128
                    nc.sync.dma_start(out_d[r0:r0 + 128, :], ot[:])

    ctx_lp.__exit__(None, None, None)
    nc.compile()
    return nc


def _prepare(inputs):
    """Host-side folding; returns (flags, x, shared input map template)."""
    f32 = np.float32
    x = np.asarray(inputs["x"], f32)
    g1 = np.asarray(inputs["g1"], f32)
    be1 = np.asarray(inputs["be1"], f32)
    g2 = np.asarray(inputs["g2"], f32)
    be2 = np.asarray(inputs["be2"], f32)
    Wq = np.asarray(inputs["Wq"], f32)
    Wk = np.asarray(inputs["Wk"], f32)
    Wv = np.asarray(inputs["Wv"], f32)
    Wo = np.asarray(inputs["Wo"], f32)
    bo = np.asarray(inputs["bo"], f32)
    W1 = np.asarray(inputs["W1"], f32)
    b1 = np.asarray(inputs["b1"], f32)
    W2 = np.asarray(inputs["W2"], f32)
    b2 = np.asarray(inputs["b2"], f32)

    wq = (g1[:, None] * Wq).astype(bf16)
    wk = (g1[:, None] * Wk).astype(bf16)
    wv = (g1[:, None] * Wv).astype(bf16)
    w1 = (g2[:, None] * W1).astype(bf16)
    bq = (be1 @ Wq).astype(f32).reshape(KC, 128).T.copy()
    bk = (be1 @ Wk).astype(f32).reshape(KC, 128).T.copy()
    vb = (be1 @ Wv).astype(f32)
    b1p = (b1 + be2 @ W1).astype(f32).reshape(FC, 128).T.copy()

    tri = np.triu(np.ones((128, 128), f32)).astype(bf16)
    idn = np.eye(128, dtype=f32).astype(bf16)
    sel = np.ones((1, 128), f32).astype(bf16)

    has_qkb = bool(np.any(bq)) or bool(np.any(bk))
    has_b1 = bool(np.any(b1p))
    has_vb = bool(np.any(vb))
    has_bo = bool(np.any(bo))
    has_b2 = bool(np.any(b2))
    shared = {
        "wq": np.ascontiguousarray(wq),
        "wk": np.ascontiguousarray(wk),
        "wv": np.ascontiguousarray(wv),
        "wo": np.ascontiguousarray(Wo.astype(bf16)),
        "w1": np.ascontiguousarray(w1),
        "w2": np.ascontiguousarray(W2.astype(bf16)),
        "bq": np.ascontiguousarray(bq),
        "bk": np.ascontiguousarray(bk),
        "b1p": np.ascontiguousarray(b1p),
        "tri": tri, "iden": idn, "sel2": sel,
    }
    if has_vb:
        shared["vbrow"] = vb.astype(bf16).reshape(1, C)
    if has_bo:
        shared["borow"] = bo.astype(bf16).reshape(1, C)
    if has_b2:
        shared["b2row"] = b2.astype(bf16).reshape(1, C)
    if has_vb or has_bo or has_b2:
        shared["ones1"] = np.ones((1, 128), bf16)
    return (has_qkb, has_b1, has_vb, has_bo, has_b2), x, shared


def _run(inputs, trace=False, **kw):
    flags, x, shared = _prepare(inputs)
    if flags not in _built:
        _built[flags] = _build(flags)
    nc = _built[flags]
    in_maps = []
    for c in range(NCORES):
        m = dict(shared)
        m["x"] = np.ascontiguousarray(
            x[c * BL:(c + 1) * BL].reshape(BL * T, C), dtype=np.float32)
        in_maps.append(m)
    res = bass_utils.run_bass_kernel_spmd(
        nc, in_maps, core_ids=list(range(NCORES)), trace=trace, **kw)
    outs = [res.results[c]["out"].reshape(BL, T, C) for c in range(NCORES)]
    return np.concatenate(outs, axis=0).astype(np.float32), res


def kernel(**inputs):
    out, _ = _run(inputs)
    return out
